# revision 15
# baseline (speedup 1.0000x reference)
"""Multi-head causal attention (B=4, T=2048, D=1024, H=16, HS=64) on 8 TRN2 cores.

Sharding: tensor-parallel over heads (2 heads/core) for QKV+attention, then an
AllToAll redistributes per-head context to token-parallel layout for the output
projection. The AllToAll is split into B=4 pieces (one per batch): each core
owns a 256-token slice of EVERY batch for the output projection, so piece b
can ship as soon as batch b's attention is done and its projection overlaps
batch b+1's attention.

Batch-pipelined emission: the Tile scheduler interleaves phase-A projections
of batch b+1 and phase-C output-projection groups of batch b-1 into the
PE-idle slots of batch b's attention (whose serializer is the exp on the
scalar engine). This keeps the PE dense (HAM stays warm) and hides the
QKV/output projections behind the softmax.

Per-phase notes:
  - qT/kT [ (h,e), t ] = W^T @ x^T with host-pretransposed weights/x.
  - vT is computed DIRECTLY in [t, (h,e)] layout by using the x^T tiles as the
    stationary operand (lhsT = xT[d, t-subtile], rhs = Wv[d, (h,e)]) -- no PE
    transposes and no PSUM->SBUF->PSUM shuffling.
  - scoresT [k, q]: both heads' matmuls use disjoint PE row groups
    (tile_position auto-derived from base partitions 0/64) so they run
    concurrently in the array.
  - softmax without max-subtraction (scores ~ N(0,1); exp safe in fp32),
    1/sqrt(HS) folded into the ACT scale; causal masking multiplies a triu 0/1
    mask on only the 128-wide diagonal band.
  - AV uses an ones-augmented stationary operand [v_h|1] (M=65): output row 64
    accumulates the softmax denominator for free.
  - normalization: denominator reciprocal via the custom-DVE
    reciprocal_approx_fast (no ACT table swap -- the baseline's ACT Reciprocal
    forced a ~1.3us exp<->recip table reload twice per chunk), broadcast
    across partitions via a K=1 matmul with an ones row.
"""
import numpy as np

import concourse.bass as bass
import concourse.tile as tile
from concourse import bacc, mybir
from concourse.bass_utils import run_bass_kernel_spmd

f32 = mybir.dt.float32
bf16 = mybir.dt.bfloat16

B, D, H, HS = 4, 1024, 16, 64
N_CORES = 8
HPC = H // N_CORES          # heads per core
QC = 512                    # q-chunk width
KT = 128                    # k-tile width
ND = D // 128               # din tiles

DT_NAME = "bf16"            # "bf16" | "f32"


def _np_dt(dt):
    import ml_dtypes
    return {f32: np.float32, bf16: ml_dtypes.bfloat16}[dt]


def build_nc(T=2048, dt_name=DT_NAME):
    DT = {"bf16": bf16, "f32": f32}[dt_name]
    BT = B * T
    SL = BT // N_CORES              # tokens per core in phase C
    NQC = T // QC                   # q-chunks per batch
    NTB = T // KT                   # k-tiles per batch
    HF = T // N_CORES               # tokens per core per a2a piece (=256)

    nc = bacc.Bacc("TRN2", target_bir_lowering=False, debug=False,
                   num_devices=N_CORES)

    xt_d = nc.dram_tensor("xt", [D, BT], DT, kind="ExternalInput").ap()
    wq_d = nc.dram_tensor("wq", [D, 128], DT, kind="ExternalInput").ap()
    wk_d = nc.dram_tensor("wk", [D, 128], DT, kind="ExternalInput").ap()
    wv_d = nc.dram_tensor("wv", [D, 128], DT, kind="ExternalInput").ap()
    wp_d = nc.dram_tensor("wp", [D, D], DT, kind="ExternalInput").ap()
    bp_d = nc.dram_tensor("bp", [D, 1], f32, kind="ExternalInput").ap()
    tri_d = nc.dram_tensor("triu", [128, 128], DT, kind="ExternalInput").ap()
    onesr_d = nc.dram_tensor("onesr", [65, 64], DT, kind="ExternalInput").ap()
    onesm_d = nc.dram_tensor("onesm", [128, NTB], DT,
                             kind="ExternalInput").ap()
    out_d = nc.dram_tensor("outT", [D, SL], f32, kind="ExternalOutput").ap()

    EXP = mybir.ActivationFunctionType.Exp

    with tile.TileContext(nc) as tc:
        with (
            tc.tile_pool(name="wts", bufs=1) as wts,
            tc.tile_pool(name="acts", bufs=1) as acts,
            tc.tile_pool(name="dram", bufs=1, space="DRAM") as dram,
        ):
            # a2a piece buffers: one per batch
            a2a_in = [dram.tile([N_CORES, 128, HF], DT, name=f"a2ai{b}")
                      for b in range(B)]
            a2a_out = [dram.tile([N_CORES, 128, HF], DT, name=f"a2ao{b}")
                       for b in range(B)]

            # per-batch activation tensors
            qT, kT, vA = [], [], []
            for b in range(B):
                qT.append(acts.tile([128, T], DT, name=f"qT{b}", tag=f"qT{b}"))
                kT.append(acts.tile([128, T], DT, name=f"kT{b}", tag=f"kT{b}"))
                vA.append(acts.tile([128, NTB * 130], DT, name=f"vA{b}",
                                    tag=f"vA{b}"))

            wp_sb, bp_sb = [], []
            cxs = {}

            with (
                tc.tile_pool(name="pA", bufs=2) as pA,
                tc.tile_pool(name="pB", bufs=2) as pB,
                tc.tile_pool(name="pC", bufs=2) as pC,
                tc.tile_pool(name="psP", bufs=2, space="PSUM") as psP,
                tc.tile_pool(name="psS", bufs=2, space="PSUM") as psS,
                tc.tile_pool(name="psAV", bufs=1, space="PSUM") as psAV,
            ):
                # ---- persistent loads (emission order = DMA priority:
                # wq first so the first projection can start, then the
                # first x chunk, then the rest of the weights) ----
                wq_sb, wk_sb, wv_sb = [], [], []
                for j in range(ND):
                    t = wts.tile([128, 128], DT, name=f"wq{j}", tag=f"wq{j}")
                    nc.sync.dma_start(t[:], wq_d[j * 128:(j + 1) * 128, :])
                    wq_sb.append(t)
                def x_dmas(b, ch):
                    i0 = b * T + ch * QC
                    xt_t = []
                    for j in range(ND):
                        t = pA.tile([128, QC], DT, name=f"x{j}", tag=f"x{j}",
                                    bufs=3)
                        nc.sync.dma_start(
                            t[:], xt_d[j * 128:(j + 1) * 128, i0:i0 + QC])
                        xt_t.append(t)
                    return xt_t

                def phase_a_chunk(b, ch, pre_x=None):
                    if ch == 0:
                        # ones columns of the augmented-V slots [v0|1|v1|1]
                        v3 = vA[b][:].rearrange("p (t c) -> p t c", c=130)
                        nc.vector.tensor_copy(v3[:, :, 64], onesm_sb[:])
                        nc.vector.tensor_copy(v3[:, :, 129], onesm_sb[:])
                    xt_t = pre_x if pre_x is not None else x_dmas(b, ch)
                    sl = slice(ch * QC, (ch + 1) * QC)
                    for w_sb, dst in ((wq_sb, qT[b]), (wk_sb, kT[b])):
                        pp = psP.tile([128, QC], f32, name="pp", tag="proj",
                                      bufs=2)
                        for j in range(ND):
                            nc.tensor.matmul(pp[:], w_sb[j][:], xt_t[j][:],
                                             start=(j == 0),
                                             stop=(j == ND - 1))
                        nc.scalar.copy(dst[:, sl], pp[:])
                    # vT computed directly: lhsT = x^T tile (tokens as M),
                    # rhs = Wv tile ((h,e) as N), contraction over d.
                    vp = psP.tile([128, QC], f32, name="vp", tag="proj",
                                  bufs=2)
                    for ts in range(QC // 128):
                        tsl = slice(ts * 128, (ts + 1) * 128)
                        for j in range(ND):
                            nc.tensor.matmul(vp[:, tsl], xt_t[j][:, tsl],
                                             wv_sb[j][:], start=(j == 0),
                                             stop=(j == ND - 1))
                    vp3 = vp[:].rearrange("p (ts c) -> p ts c", c=128)
                    va3 = vA[b][:].rearrange("p (t c) -> p t c",
                                             c=130)[:, ch * 4:(ch + 1) * 4, :]
                    for h in range(HPC):
                        nc.vector.tensor_copy(
                            va3[:, :, h * 65:h * 65 + 64],
                            vp3[:, :, h * 64:(h + 1) * 64])

                def attn_chunk(b, qc, pending):
                    nj = 4 * qc + 4
                    av = [psAV.tile([65, QC], f32, name=f"av{h}",
                                    tag=f"av{h}", bufs=1)
                          for h in range(HPC)]
                    for j in range(nj):
                        if j == 2 and pending:
                            # previous chunk's normalization matmuls land a
                            # couple of slots into this chunk so the PE FIFO
                            # never waits on the DVE reciprocal chain
                            for fn in pending:
                                fn()
                            pending.clear()
                        jr = j - 4 * qc
                        off = max(jr, 0) * 128
                        w = QC - off
                        qsl = slice(qc * QC + off, (qc + 1) * QC)
                        # both heads' scores: disjoint PE row groups (base
                        # partitions 0/64) -> concurrent in the array
                        sc = psS.tile([128, 2 * QC], f32, name="scb",
                                      tag="scb", bufs=2)
                        for h in range(HPC):
                            hp = slice(h * 64, (h + 1) * 64)
                            nc.tensor.matmul(
                                sc[:, h * QC:h * QC + w],
                                kT[b][hp, j * 128:(j + 1) * 128],
                                qT[b][hp, qsl], start=True, stop=True)
                        # one exp for both heads via a strided AP
                        e = pB.tile([128, 2 * w], DT, name="exb",
                                    tag="exb", bufs=6)
                        sc3 = sc[:].rearrange("p (two q) -> p two q",
                                              two=2)[:, :, 0:w]
                        e3 = e[:].rearrange("p (two q) -> p two q", two=2)
                        nc.scalar.activation(e3, sc3, EXP,
                                             scale=1.0 / np.sqrt(HS))
                        if jr >= 0:
                            for h in range(HPC):
                                nc.vector.tensor_mul(
                                    e[:, h * w:h * w + 128],
                                    e[:, h * w:h * w + 128], triu_sb[:])
                        for h in range(HPC):
                            lhs = vA[b][:, j * 130 + h * 65:
                                        j * 130 + h * 65 + 65]
                            nc.tensor.matmul(av[h][:, off:QC], lhs,
                                             e[:, h * w:(h + 1) * w],
                                             start=(j == 0),
                                             stop=(j == nj - 1))
                    # stage normalization: PSUM evacuation + reciprocal now
                    # (frees the av slots); the bcast matmul + ctx scaling
                    # are deferred into the next chunk via `pending`
                    from concourse.dve_ops import (
                        RECIP_APPROX_FAST_CONSTS as _RC,
                        RECIPROCAL_APPROX_FAST as _RF,
                    )
                    avs_l, rec_l = [], []
                    for h in range(HPC):
                        avs = pB.tile([65, QC], f32, name=f"avs{h}",
                                      tag=f"avs{h}", bufs=2)
                        nc.vector.tensor_copy(avs[:], av[h][:])
                        # full-tile reciprocal: single-partition [1,N] APs
                        # mis-execute the custom op; rows 0-63 are unused.
                        # bf16 out feeds the bcast matmul directly.
                        rec = pB.tile([65, QC], DT, name=f"rec{h}",
                                      tag=f"rec{h}", bufs=2)
                        nc.vector._custom_dve(
                            _RF, out=rec[:], in0=avs[:], s0=_RC["s0"],
                            s1=_RC["s1"], imm2=_RC["imm2"])
                        avs_l.append(avs)
                        rec_l.append(rec)

                    def finish_norm(b=b, qc=qc, avs_l=avs_l, rec_l=rec_l):
                        for h in range(HPC):
                            bcp = psP.tile([128, QC], f32, name="bcpp",
                                           tag="proj", bufs=2)
                            nc.tensor.matmul(
                                bcp[0:64, :], onesr_sb[64:65, :],
                                rec_l[h][64:65, :], start=True, stop=True,
                                tile_position=(64, 0))
                            ctx = pB.tile([64, QC], DT, name=f"ctx{h}",
                                          tag=f"ctx{h}", bufs=2)
                            nc.vector.tensor_mul(ctx[:], avs_l[h][0:64, :],
                                                 bcp[0:64, :])
                            nc.sync.dma_start(
                                a2a_in[b][2 * qc, h * 64:(h + 1) * 64, :],
                                ctx[:, 0:HF])
                            nc.sync.dma_start(
                                a2a_in[b][2 * qc + 1,
                                          h * 64:(h + 1) * 64, :],
                                ctx[:, HF:QC])

                    pending.append(finish_norm)

                def phc_load(b):
                    cxs[b] = []
                    for j in range(ND):
                        t = pC.tile([128, HF], DT, name=f"cx{j}",
                                    tag=f"cx{j}", bufs=2)
                        nc.scalar.dma_start(t[:], a2a_out[b][j])
                        cxs[b].append(t)

                def phc_group(b, m):
                    op = psS.tile([128, 2 * QC], f32, name="op", tag="scb",
                                  bufs=2)
                    for j in range(ND):
                        nc.tensor.matmul(
                            op[0:128, 0:HF],
                            wp_sb[j][:, m * 128:(m + 1) * 128],
                            cxs[b][j][:], start=(j == 0), stop=(j == ND - 1))
                    os_ = pC.tile([128, HF], f32, name="os", tag="os",
                                  bufs=2)
                    nc.vector.tensor_scalar_add(os_[:], op[0:128, 0:HF],
                                                bp_sb[m][:])
                    nc.sync.dma_start(
                        out_d[m * 128:(m + 1) * 128, b * HF:(b + 1) * HF],
                        os_[:])

                def do_a2a(b):
                    nc.gpsimd.collective_compute(
                        "AllToAll", mybir.AluOpType.bypass,
                        replica_groups=[list(range(N_CORES))],
                        ins=[a2a_in[b].opt()], outs=[a2a_out[b].opt()])

                x0 = x_dmas(0, 0)
                for j in range(ND):
                    for lst, dd, nm in ((wk_sb, wk_d, "wk"),
                                        (wv_sb, wv_d, "wv")):
                        t = wts.tile([128, 128], DT, name=f"{nm}{j}",
                                     tag=f"{nm}{j}")
                        nc.sync.dma_start(t[:], dd[j * 128:(j + 1) * 128, :])
                        lst.append(t)
                triu_sb = wts.tile([128, 128], DT, name="triu", tag="triu")
                nc.sync.dma_start(triu_sb[:], tri_d[:])
                onesr_sb = wts.tile([65, 64], DT, name="onesr", tag="onesr")
                nc.sync.dma_start(onesr_sb[:], onesr_d[:])
                onesm_sb = wts.tile([128, NTB], DT, name="onesm", tag="onesm")
                nc.sync.dma_start(onesm_sb[:], onesm_d[:])

                with nc.named_scope("phA0"):
                    xs = {0: x0}
                    for ch in range(NQC):
                        if ch + 1 < NQC:
                            xs[ch + 1] = x_dmas(0, ch + 1)
                        phase_a_chunk(0, ch, pre_x=xs.pop(ch))
                # output-projection weights load after phase A0's x-stream
                for j in range(ND):
                    t = wts.tile([128, D], DT, name=f"wp{j}", tag=f"wp{j}")
                    nc.sync.dma_start(t[:], wp_d[j * 128:(j + 1) * 128, :])
                    wp_sb.append(t)
                for m in range(ND):
                    t = wts.tile([128, 1], f32, name=f"bp{m}", tag=f"bp{m}")
                    nc.sync.dma_start(t[:], bp_d[m * 128:(m + 1) * 128, :])
                    bp_sb.append(t)

                # phase-C of piece b-1 is emitted only from qc==2 of batch b
                # so the PE's strict FIFO never reaches those matmuls before
                # the (asynchronous) AllToAll has delivered their inputs.
                pending = []
                for b in range(B):
                    for qc in range(NQC):
                        if b + 1 < B:
                            with nc.named_scope(f"phA{b+1}"):
                                xn = x_dmas(b + 1, qc)
                        with nc.named_scope(f"phB{b}"):
                            attn_chunk(b, qc, pending)
                        if b + 1 < B:
                            with nc.named_scope(f"phA{b+1}"):
                                phase_a_chunk(b + 1, qc, pre_x=xn)
                        if b >= 1 and qc >= 2:
                            with nc.named_scope(f"phC{b-1}"):
                                if qc == 2:
                                    phc_load(b - 1)
                                for m in range(4):
                                    phc_group(b - 1, (qc - 2) * 4 + m)
                    # the batch's last chunk must normalize + stage before
                    # its a2a piece ships
                    with nc.named_scope(f"phB{b}"):
                        for fn in pending:
                            fn()
                        pending.clear()
                    do_a2a(b)
                with nc.named_scope(f"phC{B-1}"):
                    phc_load(B - 1)
                    for m in range(ND):
                        phc_group(B - 1, m)

    nc.compile()
    return nc


def prep_inputs(x, Wq, Wk, Wv, Wp, bp, T, dt_name=DT_NAME):
    """Host-side sharding/layout prep. Returns in_maps for the 8 cores."""
    DT = {"bf16": bf16, "f32": f32}[dt_name]
    ndt = _np_dt(DT)
    BT = B * T
    NTB = T // KT

    x = np.asarray(x, np.float32)
    Wq = np.asarray(Wq, np.float32)
    Wk = np.asarray(Wk, np.float32)
    Wv = np.asarray(Wv, np.float32)
    Wp = np.asarray(Wp, np.float32)
    bp = np.asarray(bp, np.float32)

    xt = np.ascontiguousarray(x.reshape(BT, D).T).astype(ndt)
    wp = np.ascontiguousarray(Wp.T).astype(ndt)
    bpc = np.ascontiguousarray(bp.reshape(D, 1))
    triu = np.triu(np.ones((128, 128), np.float32)).astype(ndt)
    onesr = np.ones((65, 64), np.float32).astype(ndt)
    onesm = np.ones((128, NTB), np.float32).astype(ndt)

    def wslice(W, c):
        # [H, D, HS] heads 2c,2c+1 -> [D, 128] as [d, (h_local, e)]
        return np.ascontiguousarray(
            W[2 * c:2 * c + 2].transpose(1, 0, 2).reshape(D, 2 * HS)
        ).astype(ndt)

    in_maps = []
    for c in range(N_CORES):
        in_maps.append({
            "xt": xt, "wq": wslice(Wq, c), "wk": wslice(Wk, c),
            "wv": wslice(Wv, c), "wp": wp, "bp": bpc,
            "triu": triu, "onesr": onesr, "onesm": onesm,
        })
    return in_maps


_NC_CACHE = {}


def kernel(x, Wq, Wk, Wv, Wp, bp):
    T = np.asarray(x).shape[1]
    key = (T, DT_NAME)
    if key not in _NC_CACHE:
        _NC_CACHE[key] = build_nc(T, DT_NAME)
    nc = _NC_CACHE[key]
    in_maps = prep_inputs(x, Wq, Wk, Wv, Wp, bp, T, DT_NAME)
    res = run_bass_kernel_spmd(nc, in_maps, list(range(N_CORES)))
    HF = T // N_CORES
    # core d, col c (c = b*HF + i)  <->  global token b*T + d*HF + i
    per_core = np.stack([res.results[c]["outT"].T for c in range(N_CORES)])
    per_core = per_core.reshape(N_CORES, B, HF, D).transpose(1, 0, 2, 3)
    return np.ascontiguousarray(
        per_core.reshape(B, T, D).astype(np.float32))


# revision 16
# speedup vs baseline: 1.0490x; 1.0490x over previous
"""Multi-head causal attention (B=4, T=2048, D=1024, H=16, HS=64) on 8 TRN2 cores.

Sharding: tensor-parallel over heads (2 heads/core) for QKV+attention, then an
AllToAll redistributes per-head context to token-parallel layout for the output
projection. The AllToAll is split into B=4 pieces (one per batch): each core
owns a 256-token slice of EVERY batch for the output projection, so piece b
can ship as soon as batch b's attention is done and its projection overlaps
batch b+1's attention.

Batch-pipelined emission: the Tile scheduler interleaves phase-A projections
of batch b+1 and phase-C output-projection groups of batch b-1 into the
PE-idle slots of batch b's attention (whose serializer is the exp on the
scalar engine). This keeps the PE dense (HAM stays warm) and hides the
QKV/output projections behind the softmax.

Per-phase notes:
  - qT/kT [ (h,e), t ] = W^T @ x^T with host-pretransposed weights/x.
  - vT is computed DIRECTLY in [t, (h,e)] layout by using the x^T tiles as the
    stationary operand (lhsT = xT[d, t-subtile], rhs = Wv[d, (h,e)]) -- no PE
    transposes and no PSUM->SBUF->PSUM shuffling.
  - scoresT [k, q]: both heads' matmuls use disjoint PE row groups
    (tile_position auto-derived from base partitions 0/64) so they run
    concurrently in the array.
  - softmax without max-subtraction (scores ~ N(0,1); exp safe in fp32),
    1/sqrt(HS) folded into the ACT scale; causal masking multiplies a triu 0/1
    mask on only the 128-wide diagonal band.
  - AV uses an ones-augmented stationary operand [v_h|1] (M=65): output row 64
    accumulates the softmax denominator for free.
  - normalization: denominator reciprocal via the custom-DVE
    reciprocal_approx_fast (no ACT table swap -- the baseline's ACT Reciprocal
    forced a ~1.3us exp<->recip table reload twice per chunk), broadcast
    across partitions via a K=1 matmul with an ones row.
"""
import numpy as np

import concourse.bass as bass
import concourse.tile as tile
from concourse import bacc, mybir
from concourse.bass_utils import run_bass_kernel_spmd

f32 = mybir.dt.float32
bf16 = mybir.dt.bfloat16

B, D, H, HS = 4, 1024, 16, 64
N_CORES = 8
HPC = H // N_CORES          # heads per core
QC = 512                    # q-chunk width
KT = 128                    # k-tile width
ND = D // 128               # din tiles

DT_NAME = "bf16"            # "bf16" | "f32"


def _np_dt(dt):
    import ml_dtypes
    return {f32: np.float32, bf16: ml_dtypes.bfloat16}[dt]


def build_nc(T=2048, dt_name=DT_NAME):
    DT = {"bf16": bf16, "f32": f32}[dt_name]
    BT = B * T
    SL = BT // N_CORES              # tokens per core in phase C
    NQC = T // QC                   # q-chunks per batch
    NTB = T // KT                   # k-tiles per batch
    HF = T // N_CORES               # tokens per core per a2a piece (=256)

    nc = bacc.Bacc("TRN2", target_bir_lowering=False, debug=False,
                   num_devices=N_CORES)

    xt_d = nc.dram_tensor("xt", [D, BT], DT, kind="ExternalInput").ap()
    wq_d = nc.dram_tensor("wq", [D, 128], DT, kind="ExternalInput").ap()
    wk_d = nc.dram_tensor("wk", [D, 128], DT, kind="ExternalInput").ap()
    wv_d = nc.dram_tensor("wv", [D, 128], DT, kind="ExternalInput").ap()
    wp_d = nc.dram_tensor("wp", [D, D], DT, kind="ExternalInput").ap()
    bp_d = nc.dram_tensor("bp", [D, 1], f32, kind="ExternalInput").ap()
    tri_d = nc.dram_tensor("triu", [128, 128], DT, kind="ExternalInput").ap()
    onesr_d = nc.dram_tensor("onesr", [65, 64], DT, kind="ExternalInput").ap()
    onesm_d = nc.dram_tensor("onesm", [128, NTB], DT,
                             kind="ExternalInput").ap()
    out_d = nc.dram_tensor("outT", [D, SL], f32, kind="ExternalOutput").ap()

    EXP = mybir.ActivationFunctionType.Exp

    with tile.TileContext(nc) as tc:
        with (
            tc.tile_pool(name="wts", bufs=1) as wts,
            tc.tile_pool(name="acts", bufs=1) as acts,
            tc.tile_pool(name="dram", bufs=1, space="DRAM") as dram,
        ):
            # a2a piece buffers: one per batch
            a2a_in = [dram.tile([N_CORES, 128, HF], DT, name=f"a2ai{b}")
                      for b in range(B)]
            a2a_out = [dram.tile([N_CORES, 128, HF], DT, name=f"a2ao{b}")
                       for b in range(B)]

            # per-batch activation tensors
            qT, kT, vA = [], [], []
            for b in range(B):
                qT.append(acts.tile([128, T], DT, name=f"qT{b}", tag=f"qT{b}"))
                kT.append(acts.tile([128, T], DT, name=f"kT{b}", tag=f"kT{b}"))
                vA.append(acts.tile([128, NTB * 130], DT, name=f"vA{b}",
                                    tag=f"vA{b}"))

            wp_sb, bp_sb = [], []
            cxs = {}

            with (
                tc.tile_pool(name="pA", bufs=2) as pA,
                tc.tile_pool(name="pB", bufs=2) as pB,
                tc.tile_pool(name="pC", bufs=2) as pC,
                tc.tile_pool(name="psP", bufs=2, space="PSUM") as psP,
                tc.tile_pool(name="psS", bufs=2, space="PSUM") as psS,
                tc.tile_pool(name="psAV", bufs=1, space="PSUM") as psAV,
            ):
                # ---- persistent loads (emission order = DMA priority:
                # wq first so the first projection can start, then the
                # first x chunk, then the rest of the weights) ----
                wq_sb, wk_sb, wv_sb = [], [], []
                for j in range(ND):
                    t = wts.tile([128, 128], DT, name=f"wq{j}", tag=f"wq{j}")
                    nc.sync.dma_start(t[:], wq_d[j * 128:(j + 1) * 128, :])
                    wq_sb.append(t)
                def x_dmas(b, ch):
                    i0 = b * T + ch * QC
                    xt_t = []
                    for j in range(ND):
                        t = pA.tile([128, QC], DT, name=f"x{j}", tag=f"x{j}",
                                    bufs=3)
                        nc.sync.dma_start(
                            t[:], xt_d[j * 128:(j + 1) * 128, i0:i0 + QC])
                        xt_t.append(t)
                    return xt_t

                def phase_a_chunk(b, ch, pre_x=None):
                    if ch == 0:
                        # ones columns of the augmented-V slots [v0|1|v1|1]
                        v3 = vA[b][:].rearrange("p (t c) -> p t c", c=130)
                        nc.vector.tensor_copy(v3[:, :, 64], onesm_sb[:])
                        nc.vector.tensor_copy(v3[:, :, 129], onesm_sb[:])
                    xt_t = pre_x if pre_x is not None else x_dmas(b, ch)
                    sl = slice(ch * QC, (ch + 1) * QC)
                    for w_sb, dst in ((wq_sb, qT[b]), (wk_sb, kT[b])):
                        pp = psP.tile([128, QC], f32, name="pp", tag="proj",
                                      bufs=2)
                        for j in range(ND):
                            nc.tensor.matmul(pp[:], w_sb[j][:], xt_t[j][:],
                                             start=(j == 0),
                                             stop=(j == ND - 1))
                        nc.scalar.copy(dst[:, sl], pp[:])
                    # vT computed directly: lhsT = x^T tile (tokens as M),
                    # rhs = Wv tile ((h,e) as N), contraction over d.
                    vp = psP.tile([128, QC], f32, name="vp", tag="proj",
                                  bufs=2)
                    for ts in range(QC // 128):
                        tsl = slice(ts * 128, (ts + 1) * 128)
                        for j in range(ND):
                            nc.tensor.matmul(vp[:, tsl], xt_t[j][:, tsl],
                                             wv_sb[j][:], start=(j == 0),
                                             stop=(j == ND - 1))
                    vp3 = vp[:].rearrange("p (ts c) -> p ts c", c=128)
                    va3 = vA[b][:].rearrange("p (t c) -> p t c",
                                             c=130)[:, ch * 4:(ch + 1) * 4, :]
                    for h in range(HPC):
                        nc.vector.tensor_copy(
                            va3[:, :, h * 65:h * 65 + 64],
                            vp3[:, :, h * 64:(h + 1) * 64])

                def attn_chunk(b, qc, pending):
                    nj = 4 * qc + 4
                    av = [psAV.tile([65, QC], f32, name=f"av{h}",
                                    tag=f"av{h}", bufs=1)
                          for h in range(HPC)]
                    for j in range(nj):
                        if j == 2 and pending:
                            # previous chunk's normalization matmuls land a
                            # couple of slots into this chunk so the PE FIFO
                            # never waits on the DVE reciprocal chain
                            for fn in pending:
                                fn()
                            pending.clear()
                        jr = j - 4 * qc
                        off = max(jr, 0) * 128
                        w = QC - off
                        qsl = slice(qc * QC + off, (qc + 1) * QC)
                        # both heads' scores: disjoint PE row groups (base
                        # partitions 0/64) -> concurrent in the array
                        sc = psS.tile([128, 2 * QC], f32, name="scb",
                                      tag="scb", bufs=2)
                        for h in range(HPC):
                            hp = slice(h * 64, (h + 1) * 64)
                            nc.tensor.matmul(
                                sc[:, h * QC:h * QC + w],
                                kT[b][hp, j * 128:(j + 1) * 128],
                                qT[b][hp, qsl], start=True, stop=True)
                        # one exp for both heads via a strided AP
                        e = pB.tile([128, 2 * w], DT, name="exb",
                                    tag="exb", bufs=6)
                        sc3 = sc[:].rearrange("p (two q) -> p two q",
                                              two=2)[:, :, 0:w]
                        e3 = e[:].rearrange("p (two q) -> p two q", two=2)
                        nc.scalar.activation(e3, sc3, EXP,
                                             scale=1.0 / np.sqrt(HS))
                        if jr >= 0:
                            for h in range(HPC):
                                nc.vector.tensor_mul(
                                    e[:, h * w:h * w + 128],
                                    e[:, h * w:h * w + 128], triu_sb[:])
                        for h in range(HPC):
                            lhs = vA[b][:, j * 130 + h * 65:
                                        j * 130 + h * 65 + 65]
                            nc.tensor.matmul(av[h][:, off:QC], lhs,
                                             e[:, h * w:(h + 1) * w],
                                             start=(j == 0),
                                             stop=(j == nj - 1))
                    # stage normalization: PSUM evacuation + reciprocal now
                    # (frees the av slots); the bcast matmul + ctx scaling
                    # are deferred into the next chunk via `pending`
                    from concourse.dve_ops import (
                        RECIP_APPROX_FAST_CONSTS as _RC,
                        RECIPROCAL_APPROX_FAST as _RF,
                    )
                    avs_l, rec_l = [], []
                    for h in range(HPC):
                        avs = pB.tile([65, QC], f32, name=f"avs{h}",
                                      tag=f"avs{h}", bufs=2)
                        nc.vector.tensor_copy(avs[:], av[h][:])
                        # full-tile reciprocal: single-partition [1,N] APs
                        # mis-execute the custom op; rows 0-63 are unused.
                        # bf16 out feeds the bcast matmul directly.
                        rec = pB.tile([65, QC], DT, name=f"rec{h}",
                                      tag=f"rec{h}", bufs=2)
                        nc.vector._custom_dve(
                            _RF, out=rec[:], in0=avs[:], s0=_RC["s0"],
                            s1=_RC["s1"], imm2=_RC["imm2"])
                        avs_l.append(avs)
                        rec_l.append(rec)

                    def finish_norm(b=b, qc=qc, avs_l=avs_l, rec_l=rec_l):
                        for h in range(HPC):
                            bcp = psP.tile([128, QC], f32, name="bcpp",
                                           tag="proj", bufs=2)
                            nc.tensor.matmul(
                                bcp[0:64, :], onesr_sb[64:65, :],
                                rec_l[h][64:65, :], start=True, stop=True,
                                tile_position=(64, 0))
                            ctx = pB.tile([64, QC], DT, name=f"ctx{h}",
                                          tag=f"ctx{h}", bufs=2)
                            nc.vector.tensor_mul(ctx[:], avs_l[h][0:64, :],
                                                 bcp[0:64, :])
                            nc.sync.dma_start(
                                a2a_in[b][2 * qc, h * 64:(h + 1) * 64, :],
                                ctx[:, 0:HF])
                            nc.sync.dma_start(
                                a2a_in[b][2 * qc + 1,
                                          h * 64:(h + 1) * 64, :],
                                ctx[:, HF:QC])

                    pending.append(finish_norm)

                def phc_load(b):
                    cxs[b] = []
                    for j in range(ND):
                        t = pC.tile([128, HF], DT, name=f"cx{j}",
                                    tag=f"cx{j}", bufs=2)
                        nc.sync.dma_start(t[:], a2a_out[b][j])
                        cxs[b].append(t)

                def phc_group(b, m):
                    op = psS.tile([128, 2 * QC], f32, name="op", tag="scb",
                                  bufs=2)
                    for j in range(ND):
                        nc.tensor.matmul(
                            op[0:128, 0:HF],
                            wp_sb[j][:, m * 128:(m + 1) * 128],
                            cxs[b][j][:], start=(j == 0), stop=(j == ND - 1))
                    os_ = pC.tile([128, HF], f32, name="os", tag="os",
                                  bufs=2)
                    nc.vector.tensor_scalar_add(os_[:], op[0:128, 0:HF],
                                                bp_sb[m][:])
                    nc.sync.dma_start(
                        out_d[m * 128:(m + 1) * 128, b * HF:(b + 1) * HF],
                        os_[:])

                def do_a2a(b):
                    nc.gpsimd.collective_compute(
                        "AllToAll", mybir.AluOpType.bypass,
                        replica_groups=[list(range(N_CORES))],
                        ins=[a2a_in[b].opt()], outs=[a2a_out[b].opt()])

                x0 = x_dmas(0, 0)
                for j in range(ND):
                    for lst, dd, nm in ((wk_sb, wk_d, "wk"),
                                        (wv_sb, wv_d, "wv")):
                        t = wts.tile([128, 128], DT, name=f"{nm}{j}",
                                     tag=f"{nm}{j}")
                        nc.sync.dma_start(t[:], dd[j * 128:(j + 1) * 128, :])
                        lst.append(t)
                triu_sb = wts.tile([128, 128], DT, name="triu", tag="triu")
                nc.sync.dma_start(triu_sb[:], tri_d[:])
                onesr_sb = wts.tile([65, 64], DT, name="onesr", tag="onesr")
                nc.sync.dma_start(onesr_sb[:], onesr_d[:])
                onesm_sb = wts.tile([128, NTB], DT, name="onesm", tag="onesm")
                nc.sync.dma_start(onesm_sb[:], onesm_d[:])

                with nc.named_scope("phA0"):
                    xs = {0: x0}
                    for ch in range(NQC):
                        if ch + 1 < NQC:
                            xs[ch + 1] = x_dmas(0, ch + 1)
                        phase_a_chunk(0, ch, pre_x=xs.pop(ch))
                # output-projection weights load after phase A0's x-stream
                for j in range(ND):
                    t = wts.tile([128, D], DT, name=f"wp{j}", tag=f"wp{j}")
                    nc.sync.dma_start(t[:], wp_d[j * 128:(j + 1) * 128, :])
                    wp_sb.append(t)
                for m in range(ND):
                    t = wts.tile([128, 1], f32, name=f"bp{m}", tag=f"bp{m}")
                    nc.sync.dma_start(t[:], bp_d[m * 128:(m + 1) * 128, :])
                    bp_sb.append(t)

                # phase-C of piece b-1 is emitted only from qc==2 of batch b
                # so the PE's strict FIFO never reaches those matmuls before
                # the (asynchronous) AllToAll has delivered their inputs.
                pending = []
                for b in range(B):
                    for qc in range(NQC):
                        if b + 1 < B:
                            with nc.named_scope(f"phA{b+1}"):
                                xn = x_dmas(b + 1, qc)
                        with nc.named_scope(f"phB{b}"):
                            attn_chunk(b, qc, pending)
                        if b + 1 < B:
                            with nc.named_scope(f"phA{b+1}"):
                                phase_a_chunk(b + 1, qc, pre_x=xn)
                        if b >= 1 and qc >= 2:
                            with nc.named_scope(f"phC{b-1}"):
                                if qc == 2:
                                    phc_load(b - 1)
                                for m in range(4):
                                    phc_group(b - 1, (qc - 2) * 4 + m)
                    # the batch's last chunk must normalize + stage before
                    # its a2a piece ships
                    with nc.named_scope(f"phB{b}"):
                        for fn in pending:
                            fn()
                        pending.clear()
                    do_a2a(b)
                with nc.named_scope(f"phC{B-1}"):
                    phc_load(B - 1)
                    for m in range(ND):
                        phc_group(B - 1, m)

    nc.compile()
    return nc


def prep_inputs(x, Wq, Wk, Wv, Wp, bp, T, dt_name=DT_NAME):
    """Host-side sharding/layout prep. Returns in_maps for the 8 cores."""
    DT = {"bf16": bf16, "f32": f32}[dt_name]
    ndt = _np_dt(DT)
    BT = B * T
    NTB = T // KT

    x = np.asarray(x, np.float32)
    Wq = np.asarray(Wq, np.float32)
    Wk = np.asarray(Wk, np.float32)
    Wv = np.asarray(Wv, np.float32)
    Wp = np.asarray(Wp, np.float32)
    bp = np.asarray(bp, np.float32)

    xt = np.ascontiguousarray(x.reshape(BT, D).T).astype(ndt)
    wp = np.ascontiguousarray(Wp.T).astype(ndt)
    bpc = np.ascontiguousarray(bp.reshape(D, 1))
    triu = np.triu(np.ones((128, 128), np.float32)).astype(ndt)
    onesr = np.ones((65, 64), np.float32).astype(ndt)
    onesm = np.ones((128, NTB), np.float32).astype(ndt)

    def wslice(W, c):
        # [H, D, HS] heads 2c,2c+1 -> [D, 128] as [d, (h_local, e)]
        return np.ascontiguousarray(
            W[2 * c:2 * c + 2].transpose(1, 0, 2).reshape(D, 2 * HS)
        ).astype(ndt)

    in_maps = []
    for c in range(N_CORES):
        in_maps.append({
            "xt": xt, "wq": wslice(Wq, c), "wk": wslice(Wk, c),
            "wv": wslice(Wv, c), "wp": wp, "bp": bpc,
            "triu": triu, "onesr": onesr, "onesm": onesm,
        })
    return in_maps


_NC_CACHE = {}


def kernel(x, Wq, Wk, Wv, Wp, bp):
    T = np.asarray(x).shape[1]
    key = (T, DT_NAME)
    if key not in _NC_CACHE:
        _NC_CACHE[key] = build_nc(T, DT_NAME)
    nc = _NC_CACHE[key]
    in_maps = prep_inputs(x, Wq, Wk, Wv, Wp, bp, T, DT_NAME)
    res = run_bass_kernel_spmd(nc, in_maps, list(range(N_CORES)))
    HF = T // N_CORES
    # core d, col c (c = b*HF + i)  <->  global token b*T + d*HF + i
    per_core = np.stack([res.results[c]["outT"].T for c in range(N_CORES)])
    per_core = per_core.reshape(N_CORES, B, HF, D).transpose(1, 0, 2, 3)
    return np.ascontiguousarray(
        per_core.reshape(B, T, D).astype(np.float32))


# revision 17
# speedup vs baseline: 1.0556x; 1.0063x over previous
"""Multi-head causal attention (B=4, T=2048, D=1024, H=16, HS=64) on 8 TRN2 cores.

Sharding: tensor-parallel over heads (2 heads/core) for QKV+attention, then an
AllToAll redistributes per-head context to token-parallel layout for the output
projection. The AllToAll is split into B=4 pieces (one per batch): each core
owns a 256-token slice of EVERY batch for the output projection, so piece b
can ship as soon as batch b's attention is done and its projection overlaps
batch b+1's attention.

Batch-pipelined emission: the Tile scheduler interleaves phase-A projections
of batch b+1 and phase-C output-projection groups of batch b-1 into the
PE-idle slots of batch b's attention (whose serializer is the exp on the
scalar engine). This keeps the PE dense (HAM stays warm) and hides the
QKV/output projections behind the softmax.

Per-phase notes:
  - qT/kT [ (h,e), t ] = W^T @ x^T with host-pretransposed weights/x.
  - vT is computed DIRECTLY in [t, (h,e)] layout by using the x^T tiles as the
    stationary operand (lhsT = xT[d, t-subtile], rhs = Wv[d, (h,e)]) -- no PE
    transposes and no PSUM->SBUF->PSUM shuffling.
  - scoresT [k, q]: both heads' matmuls use disjoint PE row groups
    (tile_position auto-derived from base partitions 0/64) so they run
    concurrently in the array.
  - softmax without max-subtraction (scores ~ N(0,1); exp safe in fp32),
    1/sqrt(HS) folded into the ACT scale; causal masking multiplies a triu 0/1
    mask on only the 128-wide diagonal band.
  - AV uses an ones-augmented stationary operand [v_h|1] (M=65): output row 64
    accumulates the softmax denominator for free.
  - normalization: denominator reciprocal via the custom-DVE
    reciprocal_approx_fast (no ACT table swap -- the baseline's ACT Reciprocal
    forced a ~1.3us exp<->recip table reload twice per chunk), broadcast
    across partitions via a K=1 matmul with an ones row.
"""
import numpy as np

import concourse.bass as bass
import concourse.tile as tile
from concourse import bacc, mybir
from concourse.bass_utils import run_bass_kernel_spmd

f32 = mybir.dt.float32
bf16 = mybir.dt.bfloat16

B, D, H, HS = 4, 1024, 16, 64
N_CORES = 8
HPC = H // N_CORES          # heads per core
QC = 512                    # q-chunk width
KT = 128                    # k-tile width
ND = D // 128               # din tiles

DT_NAME = "bf16"            # "bf16" | "f32"


def _np_dt(dt):
    import ml_dtypes
    return {f32: np.float32, bf16: ml_dtypes.bfloat16}[dt]


def build_nc(T=2048, dt_name=DT_NAME):
    DT = {"bf16": bf16, "f32": f32}[dt_name]
    BT = B * T
    SL = BT // N_CORES              # tokens per core in phase C
    NQC = T // QC                   # q-chunks per batch
    NTB = T // KT                   # k-tiles per batch
    HF = T // N_CORES               # tokens per core per a2a piece (=256)

    nc = bacc.Bacc("TRN2", target_bir_lowering=False, debug=False,
                   num_devices=N_CORES)

    xt_d = nc.dram_tensor("xt", [D, BT], DT, kind="ExternalInput").ap()
    wq_d = nc.dram_tensor("wq", [D, 128], DT, kind="ExternalInput").ap()
    wk_d = nc.dram_tensor("wk", [D, 128], DT, kind="ExternalInput").ap()
    wv_d = nc.dram_tensor("wv", [D, 128], DT, kind="ExternalInput").ap()
    wp_d = nc.dram_tensor("wp", [D, D], DT, kind="ExternalInput").ap()
    bp_d = nc.dram_tensor("bp", [D, 1], f32, kind="ExternalInput").ap()
    tri_d = nc.dram_tensor("triu", [128, 128], DT, kind="ExternalInput").ap()
    onesr_d = nc.dram_tensor("onesr", [65, 64], DT, kind="ExternalInput").ap()
    onesm_d = nc.dram_tensor("onesm", [128, NTB], DT,
                             kind="ExternalInput").ap()
    out_d = nc.dram_tensor("outT", [D, SL], DT, kind="ExternalOutput").ap()

    EXP = mybir.ActivationFunctionType.Exp

    with tile.TileContext(nc) as tc:
        with (
            tc.tile_pool(name="wts", bufs=1) as wts,
            tc.tile_pool(name="acts", bufs=1) as acts,
            tc.tile_pool(name="dram", bufs=1, space="DRAM") as dram,
        ):
            # a2a piece buffers: one per batch
            a2a_in = [dram.tile([N_CORES, 128, HF], DT, name=f"a2ai{b}")
                      for b in range(B)]
            a2a_out = [dram.tile([N_CORES, 128, HF], DT, name=f"a2ao{b}")
                       for b in range(B)]

            # per-batch activation tensors
            qT, kT, vA = [], [], []
            for b in range(B):
                qT.append(acts.tile([128, T], DT, name=f"qT{b}", tag=f"qT{b}"))
                kT.append(acts.tile([128, T], DT, name=f"kT{b}", tag=f"kT{b}"))
                vA.append(acts.tile([128, NTB * 130], DT, name=f"vA{b}",
                                    tag=f"vA{b}"))

            wp_sb, bp_sb = [], []
            cxs = {}

            with (
                tc.tile_pool(name="pA", bufs=2) as pA,
                tc.tile_pool(name="pB", bufs=2) as pB,
                tc.tile_pool(name="pC", bufs=2) as pC,
                tc.tile_pool(name="psP", bufs=2, space="PSUM") as psP,
                tc.tile_pool(name="psS", bufs=2, space="PSUM") as psS,
                tc.tile_pool(name="psAV", bufs=1, space="PSUM") as psAV,
            ):
                # ---- persistent loads (emission order = DMA priority:
                # wq first so the first projection can start, then the
                # first x chunk, then the rest of the weights) ----
                wq_sb, wk_sb, wv_sb = [], [], []
                for j in range(ND):
                    t = wts.tile([128, 128], DT, name=f"wq{j}", tag=f"wq{j}")
                    nc.sync.dma_start(t[:], wq_d[j * 128:(j + 1) * 128, :])
                    wq_sb.append(t)
                warm = psS.tile([128, 2 * QC], f32, name="warm",
                                tag="scb", bufs=2)
                for _ in range(44):
                    nc.tensor.matmul(warm[0:128, 0:128], wq_sb[0][:],
                                     wq_sb[0][:], start=True, stop=True)

                def x_dmas(b, ch):
                    i0 = b * T + ch * QC
                    xt_t = []
                    for j in range(ND):
                        t = pA.tile([128, QC], DT, name=f"x{j}", tag=f"x{j}",
                                    bufs=3)
                        nc.sync.dma_start(
                            t[:], xt_d[j * 128:(j + 1) * 128, i0:i0 + QC])
                        xt_t.append(t)
                    return xt_t

                def phase_a_chunk(b, ch, pre_x=None):
                    if ch == 0:
                        # ones columns of the augmented-V slots [v0|1|v1|1]
                        v3 = vA[b][:].rearrange("p (t c) -> p t c", c=130)
                        nc.vector.tensor_copy(v3[:, :, 64], onesm_sb[:])
                        nc.vector.tensor_copy(v3[:, :, 129], onesm_sb[:])
                    xt_t = pre_x if pre_x is not None else x_dmas(b, ch)
                    sl = slice(ch * QC, (ch + 1) * QC)
                    for w_sb, dst in ((wq_sb, qT[b]), (wk_sb, kT[b])):
                        pp = psP.tile([128, QC], f32, name="pp", tag="proj",
                                      bufs=2)
                        for j in range(ND):
                            nc.tensor.matmul(pp[:], w_sb[j][:], xt_t[j][:],
                                             start=(j == 0),
                                             stop=(j == ND - 1))
                        nc.vector.tensor_copy(dst[:, sl], pp[:])
                    # vT computed directly: lhsT = x^T tile (tokens as M),
                    # rhs = Wv tile ((h,e) as N), contraction over d.
                    vp = psP.tile([128, QC], f32, name="vp", tag="proj",
                                  bufs=2)
                    for ts in range(QC // 128):
                        tsl = slice(ts * 128, (ts + 1) * 128)
                        for j in range(ND):
                            nc.tensor.matmul(vp[:, tsl], xt_t[j][:, tsl],
                                             wv_sb[j][:], start=(j == 0),
                                             stop=(j == ND - 1))
                    vp3 = vp[:].rearrange("p (ts c) -> p ts c", c=128)
                    va3 = vA[b][:].rearrange("p (t c) -> p t c",
                                             c=130)[:, ch * 4:(ch + 1) * 4, :]
                    for h in range(HPC):
                        nc.vector.tensor_copy(
                            va3[:, :, h * 65:h * 65 + 64],
                            vp3[:, :, h * 64:(h + 1) * 64])

                def attn_chunk(b, qc, pending):
                    nj = 4 * qc + 4
                    av = [psAV.tile([65, QC], f32, name=f"av{h}",
                                    tag=f"av{h}", bufs=1)
                          for h in range(HPC)]
                    for j in range(nj):
                        if j == 2 and pending:
                            # previous chunk's normalization matmuls land a
                            # couple of slots into this chunk so the PE FIFO
                            # never waits on the DVE reciprocal chain
                            for fn in pending:
                                fn()
                            pending.clear()
                        jr = j - 4 * qc
                        off = max(jr, 0) * 128
                        w = QC - off
                        qsl = slice(qc * QC + off, (qc + 1) * QC)
                        # both heads' scores: disjoint PE row groups (base
                        # partitions 0/64) -> concurrent in the array
                        sc = psS.tile([128, 2 * QC], f32, name="scb",
                                      tag="scb", bufs=2)
                        for h in range(HPC):
                            hp = slice(h * 64, (h + 1) * 64)
                            nc.tensor.matmul(
                                sc[:, h * QC:h * QC + w],
                                kT[b][hp, j * 128:(j + 1) * 128],
                                qT[b][hp, qsl], start=True, stop=True)
                        # one exp for both heads via a strided AP
                        e = pB.tile([128, 2 * w], DT, name="exb",
                                    tag="exb", bufs=6)
                        sc3 = sc[:].rearrange("p (two q) -> p two q",
                                              two=2)[:, :, 0:w]
                        e3 = e[:].rearrange("p (two q) -> p two q", two=2)
                        nc.scalar.activation(e3, sc3, EXP,
                                             scale=1.0 / np.sqrt(HS))
                        if jr >= 0:
                            for h in range(HPC):
                                nc.vector.tensor_mul(
                                    e[:, h * w:h * w + 128],
                                    e[:, h * w:h * w + 128], triu_sb[:])
                        for h in range(HPC):
                            lhs = vA[b][:, j * 130 + h * 65:
                                        j * 130 + h * 65 + 65]
                            nc.tensor.matmul(av[h][:, off:QC], lhs,
                                             e[:, h * w:(h + 1) * w],
                                             start=(j == 0),
                                             stop=(j == nj - 1))
                    # stage normalization: PSUM evacuation + reciprocal now
                    # (frees the av slots); the bcast matmul + ctx scaling
                    # are deferred into the next chunk via `pending`
                    from concourse.dve_ops import (
                        RECIP_APPROX_FAST_CONSTS as _RC,
                        RECIPROCAL_APPROX_FAST as _RF,
                    )
                    avs_l, rec_l = [], []
                    for h in range(HPC):
                        avs = pB.tile([65, QC], f32, name=f"avs{h}",
                                      tag=f"avs{h}", bufs=2)
                        nc.vector.tensor_copy(avs[:], av[h][:])
                        # full-tile reciprocal: single-partition [1,N] APs
                        # mis-execute the custom op; rows 0-63 are unused.
                        # bf16 out feeds the bcast matmul directly.
                        rec = pB.tile([65, QC], DT, name=f"rec{h}",
                                      tag=f"rec{h}", bufs=2)
                        nc.vector._custom_dve(
                            _RF, out=rec[:], in0=avs[:], s0=_RC["s0"],
                            s1=_RC["s1"], imm2=_RC["imm2"])
                        avs_l.append(avs)
                        rec_l.append(rec)

                    def finish_norm(b=b, qc=qc, avs_l=avs_l, rec_l=rec_l):
                        for h in range(HPC):
                            bcp = psP.tile([128, QC], f32, name="bcpp",
                                           tag="proj", bufs=2)
                            nc.tensor.matmul(
                                bcp[0:64, :], onesr_sb[64:65, :],
                                rec_l[h][64:65, :], start=True, stop=True,
                                tile_position=(64, 0))
                            ctx = pB.tile([64, QC], DT, name=f"ctx{h}",
                                          tag=f"ctx{h}", bufs=2)
                            nc.vector.tensor_mul(ctx[:], avs_l[h][0:64, :],
                                                 bcp[0:64, :])
                            nc.sync.dma_start(
                                a2a_in[b][2 * qc, h * 64:(h + 1) * 64, :],
                                ctx[:, 0:HF])
                            nc.sync.dma_start(
                                a2a_in[b][2 * qc + 1,
                                          h * 64:(h + 1) * 64, :],
                                ctx[:, HF:QC])

                    pending.append(finish_norm)

                def phc_load(b):
                    cxs[b] = []
                    for j in range(ND):
                        t = pC.tile([128, HF], DT, name=f"cx{j}",
                                    tag=f"cx{j}", bufs=2)
                        nc.sync.dma_start(t[:], a2a_out[b][j])
                        cxs[b].append(t)

                def phc_group(b, m):
                    op = psS.tile([128, 2 * QC], f32, name="op", tag="scb",
                                  bufs=2)
                    for j in range(ND):
                        nc.tensor.matmul(
                            op[0:128, 0:HF],
                            wp_sb[j][:, m * 128:(m + 1) * 128],
                            cxs[b][j][:], start=(j == 0), stop=(j == ND - 1))
                    os_ = pC.tile([128, HF], DT, name="os", tag="os",
                                  bufs=2)
                    nc.vector.tensor_scalar_add(os_[:], op[0:128, 0:HF],
                                                bp_sb[m][:])
                    nc.sync.dma_start(
                        out_d[m * 128:(m + 1) * 128, b * HF:(b + 1) * HF],
                        os_[:])

                def do_a2a(b):
                    nc.gpsimd.collective_compute(
                        "AllToAll", mybir.AluOpType.bypass,
                        replica_groups=[list(range(N_CORES))],
                        ins=[a2a_in[b].opt()], outs=[a2a_out[b].opt()])

                x0 = x_dmas(0, 0)
                for j in range(ND):
                    for lst, dd, nm in ((wk_sb, wk_d, "wk"),
                                        (wv_sb, wv_d, "wv")):
                        t = wts.tile([128, 128], DT, name=f"{nm}{j}",
                                     tag=f"{nm}{j}")
                        nc.sync.dma_start(t[:], dd[j * 128:(j + 1) * 128, :])
                        lst.append(t)
                triu_sb = wts.tile([128, 128], DT, name="triu", tag="triu")
                nc.sync.dma_start(triu_sb[:], tri_d[:])
                onesr_sb = wts.tile([65, 64], DT, name="onesr", tag="onesr")
                nc.sync.dma_start(onesr_sb[:], onesr_d[:])
                onesm_sb = wts.tile([128, NTB], DT, name="onesm", tag="onesm")
                nc.sync.dma_start(onesm_sb[:], onesm_d[:])

                with nc.named_scope("phA0"):
                    xs = {0: x0}
                    for ch in range(NQC):
                        if ch + 1 < NQC:
                            xs[ch + 1] = x_dmas(0, ch + 1)
                        phase_a_chunk(0, ch, pre_x=xs.pop(ch))
                # output-projection weights load after phase A0's x-stream
                for j in range(ND):
                    t = wts.tile([128, D], DT, name=f"wp{j}", tag=f"wp{j}")
                    nc.sync.dma_start(t[:], wp_d[j * 128:(j + 1) * 128, :])
                    wp_sb.append(t)
                for m in range(ND):
                    t = wts.tile([128, 1], f32, name=f"bp{m}", tag=f"bp{m}")
                    nc.sync.dma_start(t[:], bp_d[m * 128:(m + 1) * 128, :])
                    bp_sb.append(t)

                # phase-C of piece b-1 is emitted only from qc==2 of batch b
                # so the PE's strict FIFO never reaches those matmuls before
                # the (asynchronous) AllToAll has delivered their inputs.
                pending = []
                for b in range(B):
                    for qc in range(NQC):
                        if b + 1 < B:
                            with nc.named_scope(f"phA{b+1}"):
                                xn = x_dmas(b + 1, qc)
                        with nc.named_scope(f"phB{b}"):
                            attn_chunk(b, qc, pending)
                        if b + 1 < B:
                            with nc.named_scope(f"phA{b+1}"):
                                phase_a_chunk(b + 1, qc, pre_x=xn)
                        if b >= 1 and qc >= 2:
                            with nc.named_scope(f"phC{b-1}"):
                                if qc == 2:
                                    phc_load(b - 1)
                                for m in range(4):
                                    phc_group(b - 1, (qc - 2) * 4 + m)
                    # the batch's last chunk must normalize + stage before
                    # its a2a piece ships
                    with nc.named_scope(f"phB{b}"):
                        for fn in pending:
                            fn()
                        pending.clear()
                    do_a2a(b)
                with nc.named_scope(f"phC{B-1}"):
                    phc_load(B - 1)
                    for m in range(ND):
                        phc_group(B - 1, m)

    nc.compile()
    return nc


def prep_inputs(x, Wq, Wk, Wv, Wp, bp, T, dt_name=DT_NAME):
    """Host-side sharding/layout prep. Returns in_maps for the 8 cores."""
    DT = {"bf16": bf16, "f32": f32}[dt_name]
    ndt = _np_dt(DT)
    BT = B * T
    NTB = T // KT

    x = np.asarray(x, np.float32)
    Wq = np.asarray(Wq, np.float32)
    Wk = np.asarray(Wk, np.float32)
    Wv = np.asarray(Wv, np.float32)
    Wp = np.asarray(Wp, np.float32)
    bp = np.asarray(bp, np.float32)

    xt = np.ascontiguousarray(x.reshape(BT, D).T).astype(ndt)
    wp = np.ascontiguousarray(Wp.T).astype(ndt)
    bpc = np.ascontiguousarray(bp.reshape(D, 1))
    triu = np.triu(np.ones((128, 128), np.float32)).astype(ndt)
    onesr = np.ones((65, 64), np.float32).astype(ndt)
    onesm = np.ones((128, NTB), np.float32).astype(ndt)

    def wslice(W, c):
        # [H, D, HS] heads 2c,2c+1 -> [D, 128] as [d, (h_local, e)]
        return np.ascontiguousarray(
            W[2 * c:2 * c + 2].transpose(1, 0, 2).reshape(D, 2 * HS)
        ).astype(ndt)

    in_maps = []
    for c in range(N_CORES):
        in_maps.append({
            "xt": xt, "wq": wslice(Wq, c), "wk": wslice(Wk, c),
            "wv": wslice(Wv, c), "wp": wp, "bp": bpc,
            "triu": triu, "onesr": onesr, "onesm": onesm,
        })
    return in_maps


_NC_CACHE = {}


def kernel(x, Wq, Wk, Wv, Wp, bp):
    T = np.asarray(x).shape[1]
    key = (T, DT_NAME)
    if key not in _NC_CACHE:
        _NC_CACHE[key] = build_nc(T, DT_NAME)
    nc = _NC_CACHE[key]
    in_maps = prep_inputs(x, Wq, Wk, Wv, Wp, bp, T, DT_NAME)
    res = run_bass_kernel_spmd(nc, in_maps, list(range(N_CORES)))
    HF = T // N_CORES
    # core d, col c (c = b*HF + i)  <->  global token b*T + d*HF + i
    per_core = np.stack([res.results[c]["outT"].T for c in range(N_CORES)])
    per_core = per_core.reshape(N_CORES, B, HF, D).transpose(1, 0, 2, 3)
    return np.ascontiguousarray(
        per_core.reshape(B, T, D).astype(np.float32))
